# revision 6
# baseline (speedup 1.0000x reference)
"""RoIAlign crop+resize kernel v2 for Trainium2 (8 NeuronCores).

Descriptor-economy redesign (v1 was gather-descriptor-bound at ~12.5ns/desc):
  - Image stored block-channel-interleaved: imgX[v, ub, c, :] =
    image[c, v, ub*128:(ub+1)*128], flattened. One dma_gather per box fetches
    all 3 channels: idx = v*16 + ub0 (fits int16 exactly), elem = 3*nu*128.
  - Per-box orientation: gather from the transposed image when the span along
    y is narrower than along x. Slots are orientation-homogeneous so the SPMD
    program is shared. The resample along the span axis (u) is a banded
    matmul; along the index axis (v) a DVE/ACT lerp.
  - Banded Wu: each 128-wide output column half contracts only its diagonal
    band of 128-element chunks.
  - Lerp split: ACT does tmp = wv*psB, DVE does out = (1-wv)*psA + tmp.
  - idx/wv preloaded in one DMA each; weights and output per-partition
    contiguous; output bf16, two slots per DMA.
  - timg triple-buffered (fixed max-size tiles: varying sizes race at
    depth 3) and stage quad-buffered, so gathers and stage->HBM writes
    keep the shared SDMA engines packed while a big box's matmul/lerp
    chain drains (-5.5% vs double-buffering).
  - timg triple-buffered and stage quad-buffered so gathers and the
    stage->HBM writes keep the shared SDMA engines packed while a big
    box's matmul/lerp chain drains (-5% vs double-buffering).
"""

import os
import numpy as np
import ml_dtypes

C, H, W, N, S = 3, 2048, 2048, 512, 256
NCORES = 8
NB = N // NCORES          # 64 boxes per core
BLK = 128
NBLK = W // BLK           # 16
TRIP = 3 * BLK            # 384: one x-block of all 3 channels
MAXNU = 17                # max span blocks (2048/128 + 1)
IMGLEN = H * NBLK * TRIP + 3 * MAXNU * BLK  # 12,582,912 + span pad
NIDX = 2 * S              # 512 gather indices per box (V0 rows + V1 rows)
FOUT = C * 2 * S          # 3072 output cols per slot per partition
BF16 = ml_dtypes.bfloat16

_NC_CACHE = {}


def _axis_coords(lo, size, s):
    scale = size.astype(np.float32) / np.float32(s)
    src = (np.arange(s, dtype=np.float32)[None, :] + np.float32(0.5)) * scale[:, None] \
        - np.float32(0.5)
    max_i = np.maximum(size[:, None] - 1, 0)
    src = np.clip(src, np.float32(0.0), max_i.astype(np.float32))
    i0 = np.floor(src).astype(np.int32)
    i1 = np.minimum(i0 + 1, max_i)
    w = src - i0.astype(np.float32)
    return lo[:, None] + i0, lo[:, None] + i1, w


def _interleave(img3):
    """(3, 2048, 2048) f32 -> block-channel-interleaved flat bf16 (padded)."""
    a = img3.reshape(C, H, NBLK, BLK).transpose(1, 2, 0, 3)
    buf = np.zeros(IMGLEN, dtype=BF16)
    buf[:H * NBLK * TRIP] = np.ascontiguousarray(a, dtype=BF16).ravel()
    return buf


def _prep(image, boxes):
    boxes = boxes.astype(np.int32)
    x1 = np.clip(boxes[:, 0], 0, W); y1 = np.clip(boxes[:, 1], 0, H)
    x2 = np.clip(boxes[:, 2], 0, W); y2 = np.clip(boxes[:, 3], 0, H)
    bw = x2 - x1; bh = y2 - y1
    valid = (bw > 0) & (bh > 0)

    def span_blocks(lo, hi):
        b0 = np.minimum(np.maximum(lo, 0), W - 1) // BLK
        n = np.where(valid, (hi - b0 * BLK + BLK - 1) // BLK, 1)
        return np.maximum(n, 1).astype(np.int64)

    nxb = span_blocks(x1, x2)
    nyb = span_blocks(y1, y2)

    # orientation: flip (gather from transposed image) when y-span narrower
    flip = valid & (nyb < nxb)
    nflip = int(flip.sum())
    drop = nflip % 8
    if drop:
        cand = np.where(flip)[0]
        benefit = (nxb - nyb)[cand]
        for i in np.argsort(benefit)[:drop]:
            flip[cand[i]] = False
    nu = np.where(flip, nyb, nxb)

    # orientation-homogeneous slots: flipped boxes first, each group sorted
    # by span desc and dealt round-robin across the 8 cores
    fidx = np.where(flip)[0]
    nidx = np.where(~flip)[0]
    fidx = fidx[np.argsort(-nu[fidx], kind="stable")]
    nidx = nidx[np.argsort(-nu[nidx], kind="stable")]
    order = np.concatenate([fidx, nidx])
    asg = order.reshape(NB, NCORES)              # [slot, core] -> box
    flip_slot = flip[asg[:, 0]].copy()
    nu_slot = nu[asg].max(axis=1).astype(np.int64)   # [slot]

    # per-box sampling geometry in gather space (u = span axis, v = idx axis)
    U0 = np.zeros((N, S), np.int32); U1 = np.zeros((N, S), np.int32)
    wu = np.zeros((N, S), np.float32)
    V0 = np.zeros((N, S), np.int32); V1 = np.zeros((N, S), np.int32)
    wv = np.zeros((N, S), np.float32)
    u1b = np.where(flip, y1, x1); u2b = np.where(flip, y2, x2)
    v1b = np.where(flip, x1, y1); v2b = np.where(flip, x2, y2)
    su = u2b - u1b; sv = v2b - v1b
    a0, a1, aw = _axis_coords(u1b, su, S)
    b0_, b1_, bw_ = _axis_coords(v1b, sv, S)
    U0[:], U1[:], wu[:] = np.clip(a0, 0, W - 1), np.clip(a1, 0, W - 1), aw
    V0[:], V1[:], wv[:] = np.clip(b0_, 0, W - 1), np.clip(b1_, 0, W - 1), bw_

    # per-slot band structure (union over the 8 cores of each slot)
    gl = np.zeros((NB, 2), np.int64)
    gh = np.zeros((NB, 2), np.int64)
    bu0 = np.zeros(N, np.int64)
    for j in range(NB):
        ns = int(nu_slot[j])
        lo = np.full(2, 10 ** 9); hi = np.full(2, -1)
        for k in range(NCORES):
            b = int(asg[j, k])
            if not valid[b]:
                continue
            base = min(max(int(u1b[b]), 0) // BLK, NBLK - ns)
            bu0[b] = base
            ur0 = U0[b] - base * BLK
            ur1 = U1[b] - base * BLK
            for h in range(2):
                lo[h] = min(lo[h], ur0[h * 128] // BLK)
                hi[h] = max(hi[h], ur1[h * 128 + 127] // BLK)
        for h in range(2):
            if hi[h] < 0:
                lo[h], hi[h] = 0, 0
            gl[j, h], gh[j, h] = lo[h], hi[h]
    L = (gh - gl + 1).astype(np.int64)           # [slot, half] band lengths
    wcols = (L.sum(axis=1)) * BLK                # weight cols per slot
    woff = np.concatenate([[0], np.cumsum(wcols)]).astype(np.int64)
    TOTW = int(woff[-1])

    imgX = _interleave(image)
    imgXT = _interleave(np.ascontiguousarray(image.transpose(0, 2, 1)))

    in_maps = []
    for k in range(NCORES):
        idxall = np.zeros((128, NB * (NIDX // 16)), np.int16)
        wvall = np.zeros((128, NB * 4), np.float32)
        wub = np.zeros((128, TOTW), BF16)
        for j in range(NB):
            b = int(asg[j, k])
            ns = int(nu_slot[j])
            if valid[b]:
                base = int(bu0[b])
                iv = np.empty(NIDX, np.int16)
                iv[:S] = (V0[b] * NBLK + base).astype(np.int16)
                iv[S:] = (V1[b] * NBLK + base).astype(np.int16)
                idxall[:, j * 32:(j + 1) * 32] = np.tile(
                    iv.reshape(NIDX // 16, 16).T, (8, 1))
                wvall[:, j * 4 + 0] = wv[b][:128]
                wvall[:, j * 4 + 1] = wv[b][128:]
                wvall[:, j * 4 + 2] = 1.0 - wv[b][:128]
                wvall[:, j * 4 + 3] = 1.0 - wv[b][128:]
                wm = np.zeros((ns * BLK, S), np.float32)
                t = np.arange(S)
                ur0 = U0[b] - base * BLK
                ur1 = U1[b] - base * BLK
                wm[ur0, t] = 1.0 - wu[b]
                wm[ur1, t] += wu[b]
                col = int(woff[j])
                for h in range(2):
                    lh = int(L[j, h])
                    blkw = wm[int(gl[j, h]) * BLK:(int(gl[j, h]) + lh) * BLK,
                              h * 128:(h + 1) * 128]
                    # layout [p, chunk, t]: col p*? -> per-partition band
                    wub[:, col:col + lh * BLK] = blkw.reshape(
                        lh, BLK, BLK).transpose(1, 0, 2).reshape(128, lh * BLK)
                    col += lh * BLK
        m = {"imgb": imgX, "imgbt": imgXT, "wub": wub,
             "idxall": idxall, "wvall": wvall}
        in_maps.append(m)
    shape_key = (tuple(nu_slot), tuple(flip_slot.astype(int)),
                 tuple(L.ravel()), tuple(gl.ravel()), tuple(woff))
    return in_maps, asg, valid, flip_slot, nu_slot, gl, L, woff, shape_key


def _build(nu_slot, flip_slot, gl, L, woff, repeat=1,
           do_gather=True, do_mm=True, do_lerp=True, do_out=True,
           gq=2):
    import contextlib
    import concourse.bacc as bacc
    import concourse.mybir as mybir
    from concourse.tile import TileContext

    dt = mybir.dt
    nc = bacc.Bacc("TRN2", target_bir_lowering=False, debug=False,
                   enable_asserts=False, num_devices=NCORES,
                   num_swdge_queues=gq, dynamic_dma_scratch_size=32768)
    gather_counter = [0]
    imgb = nc.dram_tensor("imgb", [IMGLEN], dt.bfloat16,
                          kind="ExternalInput").ap()
    imgbt = nc.dram_tensor("imgbt", [IMGLEN], dt.bfloat16,
                           kind="ExternalInput").ap()
    TOTW = int(woff[-1])
    wub = nc.dram_tensor("wub", [128, TOTW], dt.bfloat16,
                         kind="ExternalInput").ap()
    idxall = nc.dram_tensor("idxall", [128, NB * (NIDX // 16)], dt.int16,
                            kind="ExternalInput").ap()
    wvall = nc.dram_tensor("wvall", [128, NB * 4], dt.float32,
                           kind="ExternalInput").ap()
    outp = nc.dram_tensor("out", [128, NB, FOUT], dt.bfloat16,
                          kind="ExternalOutput").ap()

    def src_ap(flip, nu, c0=0):
        base = (imgbt if flip else imgb)
        a = base[c0 * TRIP:].rearrange("(r e) -> r e", e=TRIP)
        a = a.copy()
        ap = a.ap
        ap[0] = [TRIP, H * NBLK]
        ap[-1] = [1, 3 * nu * BLK]
        a.ap = ap
        return a

    with TileContext(nc) as tc:
        with tc.tile_pool(name="pre", bufs=1) as prep, \
             tc.tile_pool(name="io", bufs=6) as iop, \
             tc.tile_pool(name="tp", bufs=2) as tpool, \
             tc.tile_pool(name="tg", bufs=3) as tgp, \
             tc.tile_pool(name="st", bufs=6) as stp, \
             tc.tile_pool(name="ps", bufs=4, space="PSUM") as psp:
            idxt = prep.tile([128, NB * (NIDX // 16)], dt.int16, tag="idx")
            nc.sync.dma_start(out=idxt[:], in_=idxall)
            wvt = prep.tile([128, NB * 4], dt.float32, tag="wv")
            nc.sync.dma_start(out=wvt[:], in_=wvall)
            with (tc.For_i(0, repeat, 1) if repeat > 1
                  else contextlib.nullcontext()):
                for j0 in range(0, NB, 2):
                    pcols = int(woff[j0 + 2] - woff[j0])
                    wut = tpool.tile([128, pcols // BLK, BLK], dt.bfloat16,
                                     tag="wu")
                    nc.sync.dma_start(
                        out=wut[:],
                        in_=wub[:, int(woff[j0]):int(woff[j0 + 2])].rearrange(
                            "p (i t) -> p i t", t=BLK))
                    stage = stp.tile([128, 2, C, 2, S], dt.bfloat16,
                                     tag="stage")
                    for jj in range(2):
                        j = j0 + jj
                        nu = int(nu_slot[j])
                        timg = tgp.tile([128, 2, 3 * nu, S], dt.bfloat16,
                                        tag="timg")
                        # split into V0/V1 halves (256 idx) on alternate
                        # queues; span chunks capped at NU_CAP per gather to
                        # fit the SWDGE descriptor ring
                        NU_CAP = 10
                        if do_gather:
                            for half in range(2):
                                c0 = 0
                                while c0 < nu:
                                    cn = min(NU_CAP, nu - c0)
                                    a = src_ap(bool(flip_slot[j]), cn, c0)
                                    nc.gpsimd.dma_gather(
                                        out_ap=timg[:, half,
                                                    3 * c0:3 * (c0 + cn), :],
                                        in_ap=a,
                                        idxs_ap=idxt[:, j * 32 + half * 16:
                                                     j * 32 + (half + 1) * 16],
                                        num_idxs=S,
                                        num_idxs_reg=S,
                                        elem_size=3 * cn * BLK,
                                        elem_step=TRIP,
                                        transpose=True,
                                        queue_num=gather_counter[0] % gq,
                                    )
                                    gather_counter[0] += 1
                                    c0 += cn
                        wbase = (int(woff[j]) - int(woff[j0])) // BLK
                        nw = pcols // BLK
                        for c in range(C):
                            if do_mm or do_lerp:
                                psA = psp.tile([128, 2, S], dt.float32,
                                               tag="psA")
                                psB = psp.tile([128, 2, S], dt.float32,
                                               tag="psB")
                            if do_mm:
                                for sh in range(2):
                                    for half, ps, hv in ((h, p, v)
                                                         for v, p in ((0, psA),
                                                                      (1, psB))
                                                         for h in range(2)):
                                        lh = int(L[j, half])
                                        g0 = int(gl[j, half])
                                        boff = wbase + (int(L[j, 0])
                                                        if half else 0)
                                        for i in range(lh):
                                            cx = g0 + i
                                            A = (timg[:, hv, 3 * cx + c,
                                                      sh * 128:(sh + 1) * 128]
                                                 if do_gather else
                                                 wut[:, (boff + i) % nw, :])
                                            nc.tensor.matmul(
                                                ps[:, sh,
                                                   half * 128:(half + 1) * 128],
                                                A,
                                                wut[:, boff + i, :],
                                                start=(i == 0),
                                                stop=(i == lh - 1))
                            elif do_lerp:
                                nc.tensor.matmul(
                                    psA[:].rearrange("p h t -> p (h t)"),
                                    wut[0:1, 0, :], wut[0:1, :4, :].rearrange(
                                        "p i t -> p (i t)"),
                                    start=True, stop=True)
                                nc.tensor.matmul(
                                    psB[:].rearrange("p h t -> p (h t)"),
                                    wut[0:1, 0, :], wut[0:1, :4, :].rearrange(
                                        "p i t -> p (i t)"),
                                    start=True, stop=True)
                            if do_lerp:
                                for sh in range(2):
                                    tmp = iop.tile([128, S], dt.bfloat16,
                                                   tag="tmp")
                                    nc.scalar.mul(
                                        tmp[:], psB[:, sh, :],
                                        wvt[:, j * 4 + sh:j * 4 + sh + 1])
                                    nc.vector.scalar_tensor_tensor(
                                        stage[:, jj, c, sh, :], psA[:, sh, :],
                                        wvt[:, j * 4 + 2 + sh:j * 4 + 3 + sh],
                                        tmp[:],
                                        mybir.AluOpType.mult,
                                        mybir.AluOpType.add)
                    if do_out:
                        if not do_lerp:
                            nc.sync.dma_start(
                                out=stage[:].rearrange("p n c h t -> p n (c h t)"),
                                in_=outp[:, j0:j0 + 2, :])
                        nc.sync.dma_start(
                            out=outp[:, j0:j0 + 2, :],
                            in_=stage[:].rearrange("p n c h t -> p n (c h t)"))
    nc.compile()
    return nc


def kernel(image, boxes, crop_size):
    from concourse.bass_utils import run_bass_kernel_spmd

    image = np.asarray(image, dtype=np.float32)
    boxes = np.asarray(boxes)
    (in_maps, asg, valid, flip_slot, nu_slot, gl, L, woff,
     shape_key) = _prep(image, boxes)

    if shape_key not in _NC_CACHE:
        _NC_CACHE.clear()
        _NC_CACHE[shape_key] = _build(nu_slot, flip_slot, gl, L, woff)
    nc = _NC_CACHE[shape_key]

    res = run_bass_kernel_spmd(nc, in_maps, list(range(NCORES)))
    global LAST_RESULTS
    LAST_RESULTS = res

    out = np.zeros((N, C, S, S), dtype=np.float32)
    fmask = flip_slot.astype(bool)
    for k in range(NCORES):
        o = np.asarray(res.results[k]["out"]).reshape(128, NB, C, 2, S)
        o = o.astype(np.float32)
        # normal: out[c, sh*128+p, t] = o[p, j, c, sh, t]
        nsl = np.where(~fmask)[0]
        if len(nsl):
            out[asg[nsl, k]] = o[:, nsl].transpose(1, 2, 3, 0, 4).reshape(
                len(nsl), C, S, S)
        # flipped: out[c, t, sh*128+p] = o[p, j, c, sh, t]
        fsl = np.where(fmask)[0]
        if len(fsl):
            out[asg[fsl, k]] = o[:, fsl].transpose(1, 2, 4, 3, 0).reshape(
                len(fsl), C, S, S)
    out[~valid] = 0.0
    return out


LAST_RESULTS = None

